# revision 16
# baseline (speedup 1.0000x reference)
"""Trainium2 Bass kernel: dense transformer block (LN -> causal MHA -> res -> LN -> MLP -> res).

Sharding: zero-communication query-parallel over 8 cores. Core c (b = c//4,
i = c%4) handles batch b's query chunks [256i, 256i+256) ("lo") and
[256(7-i), 256(8-i)) ("hi"). Every core computes K/V for its full batch
locally. Causality is exploited uniformly: the lo slot attends over keys
[0, 1024), the hi slot over [0, 2048), with host-supplied additive masks
making the program identical on all cores (SPMD requirement).

All activations are kept transposed ([d, tokens]) so the contraction dim
lands on SBUF partitions; the host transposes x on the way in and the
output on the way out. LayerNorm statistics are computed with ones-vector
matmuls on the PE; per-token scale/shift are broadcast across partitions
with rank-1/rank-2 matmuls. Softmax needs no max-subtraction (scores are
bounded: inputs are unit-scale, weights 0.02-scale), and the denominator
comes from an appended ones-column on V.
"""

import numpy as np
import ml_dtypes
from contextlib import ExitStack

import concourse.bass as bass
import concourse.mybir as mybir
import concourse.tile as tile
from concourse import bacc
from concourse.bass import ts

F32 = mybir.dt.float32
F32R = mybir.dt.float32r
BF16 = mybir.dt.bfloat16
AF = mybir.ActivationFunctionType
OP = mybir.AluOpType

B, T, D = 2, 2048, 1024
H, HD = 16, 64
DFF = 4096
P = 128
DT = D // P            # 8 d-tiles
KT = T // P            # 16 key tiles
R = 512                # query rows per core
QW = 256               # queries per slot
EA_KT = 16             # hi slot key tiles (keys [0, 2048))
EB_KT = 8              # lo slot key tiles (keys [0, 1024))
NEG = np.float32(-1e30)
N_CORES = 8


def _build_program():
    nc = bacc.Bacc(None, target_bir_lowering=False, debug=True)

    xT_d = nc.dram_tensor("xT", [D, T], F32R, kind="ExternalInput")
    xqT_d = nc.dram_tensor("xqT", [D, R], F32R, kind="ExternalInput")
    wq_d = nc.dram_tensor("wq", [D, D], BF16, kind="ExternalInput")
    wk_d = nc.dram_tensor("wk", [D, D], BF16, kind="ExternalInput")
    wv_d = nc.dram_tensor("wv", [D, D], BF16, kind="ExternalInput")
    wo_d = nc.dram_tensor("wo", [D, D], BF16, kind="ExternalInput")
    w1_d = nc.dram_tensor("w1", [D, DFF], F32R, kind="ExternalInput")
    w2_d = nc.dram_tensor("w2", [DFF, D], BF16, kind="ExternalInput")
    bq_d = nc.dram_tensor("bq", [D], F32, kind="ExternalInput")
    bk_d = nc.dram_tensor("bk", [D], F32, kind="ExternalInput")
    bv_d = nc.dram_tensor("bv", [D], F32R, kind="ExternalInput")
    bo_d = nc.dram_tensor("bo", [D], F32, kind="ExternalInput")
    b1_d = nc.dram_tensor("b1", [DFF], F32, kind="ExternalInput")
    b2_d = nc.dram_tensor("b2", [D], F32, kind="ExternalInput")
    g1_d = nc.dram_tensor("ln1gb", [2, D], F32R, kind="ExternalInput")
    g2_d = nc.dram_tensor("ln2gb", [2, D], F32R, kind="ExternalInput")
    mA_d = nc.dram_tensor("maskA", [T, QW], BF16, kind="ExternalInput")
    mB_d = nc.dram_tensor("maskB", [EB_KT * P, QW], BF16, kind="ExternalInput")
    out_d = nc.dram_tensor("outT", [D, R], F32, kind="ExternalOutput")

    xT_r = xT_d[:, :].rearrange("(t p) n -> p t n", p=P)
    out_r = out_d[:, :].rearrange("(t p) n -> p t n", p=P)

    with tile.TileContext(nc) as tc, ExitStack() as ctx:
        ctx.enter_context(nc.allow_low_precision(
            reason="float32r outputs feed fp32r matmuls by design"))
        pers = ctx.enter_context(tc.tile_pool(name="pers", bufs=1))

        # ---- constants / small persistent tiles
        ones128 = pers.tile([P, 1], F32R)
        nc.vector.memset(ones128.bitcast(F32), 1.0)
        ones_row = pers.tile([1, P], F32R)
        nc.vector.memset(ones_row.bitcast(F32), 1.0)
        eps1 = pers.tile([1, 1], F32)
        nc.vector.memset(eps1, 1e-5)

        def load_stack(t0, tag):
            st = pers.tile([2, D], F32R, tag=tag)
            nc.sync.dma_start(st, t0[:, :])
            return st

        g1 = load_stack(g1_d, 'g1')
        g2 = load_stack(g2_d, 'g2')

        def load_bias_cols(src, ntiles, tag):
            t = pers.tile([P, ntiles], F32, tag=tag)
            nc.sync.dma_start(t, src[:].rearrange("(t p) -> p t", p=P))
            return t

        bo_sb = load_bias_cols(bo_d, DT, 'bo')
        bq_sb = load_bias_cols(bq_d, DT, 'bq')
        bk_sb = load_bias_cols(bk_d, DT, 'bk')
        b2_sb = load_bias_cols(b2_d, DT, 'b2')
        b1_sb = load_bias_cols(b1_d, DFF // P, 'b1')
        bv_row = pers.tile([1, D], F32R)
        nc.sync.dma_start(bv_row, bv_d[:].rearrange("(a d) -> a d", a=1))

        # ---- persistent activations (allocated up front, stack base)
        xnT = pers.tile([P, DT, T], BF16, tag="xnT_hg")       # 32KB/p, reused as hg
        xnqT = pers.tile([P, DT, R], BF16, tag="xnq_xn2")      # 16KB/p (shared w/ xn2T f32)
        KTs = pers.tile([P, DT, T], BF16)                      # 32KB/p
        Vs = pers.tile([P, KT, H * (HD + 1)], BF16)            # 32.5KB/p
        qTs = pers.tile([P, DT, R], BF16)                      # 8KB/p
        YTs = pers.tile([P, DT, R], BF16)                      # 8KB/p
        x1T = pers.tile([P, DT, R], F32)                       # 16KB/p

        # =============== transposed layernorm helper =================
        def ln_T(get_src, ncols, gstack, write_dst, stats_f32r=True):
            nch = ncols // 512
            with (
                tc.tile_pool(name="lnw", bufs=3) as lw,
                tc.tile_pool(name="lns", bufs=1) as lsm,
                tc.tile_pool(name="lnpa", bufs=2, space="PSUM") as pa,
                tc.tile_pool(name="lnpb", bufs=2, space="PSUM") as pb,
            ):
                for ch in range(nch):
                    sum_ps = pa.tile([1, 512], F32, tag="lnsum")
                    ssq_ps = pa.tile([1, 512], F32, tag="lnssq")
                    sdt = F32R if stats_f32r else F32
                    one_l = ones128 if stats_f32r else ones128.bitcast(F32)
                    for dt_i in range(DT):
                        xs = get_src(dt_i, ch)
                        sq = lw.tile([P, 512], sdt, tag="lnsq")
                        nc.vector.tensor_mul(sq, xs, xs)
                        nc.tensor.matmul(
                            sum_ps, one_l, xs,
                            start=(dt_i == 0), stop=(dt_i == DT - 1))
                        nc.tensor.matmul(
                            ssq_ps, one_l, sq[:, :],
                            start=(dt_i == 0), stop=(dt_i == DT - 1))
                    mu = lsm.tile([1, 512], F32, tag="mu")
                    nc.vector.tensor_scalar_mul(mu, sum_ps, 1.0 / D)
                    msq = lsm.tile([1, 512], F32, tag="msq")
                    nc.vector.tensor_scalar_mul(msq, ssq_ps, 1.0 / D)
                    musq = lsm.tile([1, 512], F32, tag="musq")
                    nc.vector.tensor_mul(musq, mu, mu)
                    nc.vector.tensor_sub(msq, msq, musq)
                    sd = lsm.tile([1, 512], F32, tag="sd")
                    nc.scalar.activation(sd, msq, AF.Sqrt, bias=eps1)
                    rsd = lsm.tile([1, 512], F32R, tag="rstd")
                    nc.vector.reciprocal(rsd, sd)
                    r2 = lsm.tile([2, 512], F32R, tag="r2")
                    nc.vector.memset(r2.bitcast(F32), 1.0)
                    nc.vector.scalar_tensor_tensor(
                        r2[0:1, :], mu, -1.0, rsd, OP.mult, OP.mult)
                    for dt_i in range(DT):
                        b1p = pb.tile([P, 512], F32, tag="lnb1")
                        nc.tensor.matmul(
                            b1p, gstack[0:1, ts(dt_i, P)],
                            rsd, start=True, stop=True)
                        b2p = pb.tile([P, 512], F32, tag="lnb2")
                        nc.tensor.matmul(
                            b2p, gstack[0:2, ts(dt_i, P)],
                            r2[:, :], start=True, stop=True)
                        xs = get_src(dt_i, ch)
                        t = lw.tile([P, 512], F32, tag="lnt")
                        nc.vector.tensor_mul(t, xs, b1p)
                        nc.vector.tensor_add(write_dst(dt_i, ch), t, b2p)

        # =============== P1: LN1 over full batch (streamed) and queries ====
        with tc.tile_pool(name="xstream", bufs=3) as xsp:
            def src_xT(dt_i, ch):
                t = xsp.tile([P, 512], F32R, tag="xs")
                nc.sync.dma_start(t, xT_r[:, dt_i, ts(ch, 512)])
                return t

            ln_T(src_xT, T, g1, lambda dt_i, ch: xnT[:, dt_i, ts(ch, 512)])

        xqT_r = xqT_d[:, :].rearrange("(t p) n -> p t n", p=P)
        with tc.tile_pool(name="xqstream", bufs=3) as xqp:
            def src_xq(dt_i, ch):
                t = xqp.tile([P, 512], F32R, tag="xqs")
                nc.sync.dma_start(t, xqT_r[:, dt_i, :])
                return t

            ln_T(src_xq, R, g1,
                 lambda dt_i, ch: xnqT[:, dt_i, ts(ch, 512)])

        # =============== P2: projections ====================================
        # Q^T = Wq^T @ xnq^T  (+bq)
        with (
            tc.tile_pool(name="wqp", bufs=1) as wp,
            tc.tile_pool(name="p2ps", bufs=4, space="PSUM") as pp,
        ):
            wq_sb = wp.tile([P, DT, D], BF16)
            nc.sync.dma_start(wq_sb, wq_d[:, :].rearrange("(t p) m -> p t m", p=P))
            for dt_o in range(DT):
                ps = pp.tile([P, R], F32, tag="proj")
                for kt_i in range(DT):
                    nc.tensor.matmul(
                        ps, wq_sb[:, kt_i, ts(dt_o, P)], xnqT[:, kt_i, :],
                        start=(kt_i == 0), stop=(kt_i == DT - 1))
                nc.vector.tensor_scalar_add(qTs[:, dt_o, :], ps,
                                            bq_sb[:, dt_o:dt_o + 1])

        # K^T = Wk^T @ xn^T  (+bk)
        with (
            tc.tile_pool(name="wkp", bufs=1) as wp,
            tc.tile_pool(name="p2ps", bufs=4, space="PSUM") as pp,
        ):
            wk_sb = wp.tile([P, DT, D], BF16)
            nc.sync.dma_start(wk_sb, wk_d[:, :].rearrange("(t p) m -> p t m", p=P))
            for dt_o in range(DT):
                for chn in range(T // 512):
                    ps = pp.tile([P, 512], F32, tag="proj")
                    for kt_i in range(DT):
                        nc.tensor.matmul(
                            ps, wk_sb[:, kt_i, ts(dt_o, P)],
                            xnT[:, kt_i, ts(chn, 512)],
                            start=(kt_i == 0), stop=(kt_i == DT - 1))
                    nc.vector.tensor_scalar_add(KTs[:, dt_o, ts(chn, 512)], ps,
                                                bk_sb[:, dt_o:dt_o + 1])

        # V natural = xn @ Wv (+bv broadcast), interleaved per head with a
        # ones column at slot HD so AV matmuls also accumulate the softmax
        # denominator.
        with (
            tc.tile_pool(name="wvp", bufs=1) as wp,
            tc.tile_pool(name="p2ps", bufs=4, space="PSUM") as pp,
        ):
            bvb = wp.tile([P, D], BF16)
            for chn in range(2):
                ps = pp.tile([P, 512], F32, tag="proj")
                nc.tensor.matmul(ps, ones_row,
                                 bv_row[0:1, ts(chn, 512)],
                                 start=True, stop=True)
                nc.vector.tensor_copy(bvb[:, ts(chn, 512)], ps)
            for vch in range(2):
                wv_sb = wp.tile([P, DT, 512], BF16, tag="wv")
                nc.sync.dma_start(
                    wv_sb,
                    wv_d[:, ts(vch, 512)].rearrange("(t p) m -> p t m", p=P))
                for kto in range(KT):
                    ps = pp.tile([P, 512], F32, tag="proj")
                    for kt_i in range(DT):
                        nc.tensor.matmul(
                            ps, xnT[:, kt_i, ts(kto, P)], wv_sb[:, kt_i, :],
                            start=(kt_i == 0), stop=(kt_i == DT - 1))
                    dst = Vs[:, kto, :].rearrange(
                        "p (h c) -> p h c", c=HD + 1)[:, 8 * vch:8 * vch + 8, 0:HD]
                    nc.vector.tensor_add(
                        dst, ps.rearrange("p (h c) -> p h c", c=HD),
                        bvb[:, ts(vch, 512)].rearrange("p (h c) -> p h c", c=HD))
            ones_col = Vs[:, :, :].rearrange("p t (h c) -> p t h c", c=HD + 1)[:, :, :, HD:HD + 1]
            nc.vector.memset(ones_col, 1.0)

        # =============== P3: attention =====================================
        with (
            tc.tile_pool(name="maskp", bufs=1) as mp,
            tc.tile_pool(name="attw", bufs=3) as aw,
            tc.tile_pool(name="atts", bufs=2) as asml,
            tc.tile_pool(name="ps3s", bufs=3, space="PSUM") as ps3s,
            tc.tile_pool(name="ps3y", bufs=2, space="PSUM") as ps3y,
            tc.tile_pool(name="ps3b", bufs=1, space="PSUM") as ps3b,
        ):
            mA_sb = mp.tile([P, EA_KT, QW], BF16)
            nc.sync.dma_start(mA_sb, mA_d[:, :].rearrange("(t p) q -> p t q", p=P))
            mB_sb = mp.tile([P, EB_KT, QW], BF16)
            nc.sync.dma_start(mB_sb, mB_d[:, :].rearrange("(t p) q -> p t q", p=P))
            for h in range(H):
                base = (h % 2) * 64
                dt_h = h // 2
                for (qoff, nkt, m_sb) in ((QW, EA_KT, mA_sb), (0, EB_KT, mB_sb)):
                    y_ps = ps3y.tile([HD + 1, QW], F32, tag="ypsum")
                    for kt_i in range(nkt):
                        s_ps = ps3s.tile([P, QW], F32, tag="spsum")
                        nc.tensor.matmul(
                            s_ps, KTs[base:base + 64, dt_h, ts(kt_i, P)],
                            qTs[base:base + 64, dt_h, qoff:qoff + QW],
                            start=True, stop=True)
                        p32 = aw.tile([P, QW], F32, tag="p32")
                        nc.vector.tensor_add(p32, s_ps, m_sb[:, kt_i, :])
                        pbf = aw.tile([P, QW], BF16, tag="pbf")
                        nc.scalar.activation(pbf, p32, AF.Exp, scale=0.125)
                        nc.tensor.matmul(
                            y_ps, Vs[:, kt_i, h * (HD + 1):(h + 1) * (HD + 1)], pbf,
                            start=(kt_i == 0), stop=(kt_i == nkt - 1))
                    rec = asml.tile([1, QW], F32R, tag="rec")
                    nc.vector.reciprocal(rec, y_ps[HD:HD + 1, :])
                    b_ps = ps3b.tile([64, QW], F32, tag="bps")
                    nc.tensor.matmul(b_ps, ones_row[0:1, 0:64],
                                     rec, start=True, stop=True)
                    b_sb = aw.tile([64, QW], F32, tag="bsb")
                    nc.vector.tensor_copy(b_sb, b_ps)
                    nc.vector.tensor_mul(
                        YTs[base:base + 64, dt_h, qoff:qoff + QW],
                        y_ps[0:HD, :], b_sb)

        # =============== P4: output projection + residual ==================
        with (
            tc.tile_pool(name="wop", bufs=1) as wp,
            tc.tile_pool(name="p4ps", bufs=4, space="PSUM") as pp,
        ):
            wo_sb = wp.tile([P, DT, D], BF16)
            nc.sync.dma_start(wo_sb, wo_d[:, :].rearrange("(t p) m -> p t m", p=P))
            for dt_o in range(DT):
                ps = pp.tile([P, R], F32, tag="proj")
                for kt_i in range(DT):
                    nc.tensor.matmul(
                        ps, wo_sb[:, kt_i, ts(dt_o, P)], YTs[:, kt_i, :],
                        start=(kt_i == 0), stop=(kt_i == DT - 1))
                xq4 = wp.tile([P, R], F32R, tag="xq4")
                nc.sync.dma_start(xq4, xqT_r[:, dt_o, :])
                nc.vector.scalar_tensor_tensor(
                    x1T[:, dt_o, :], ps, bo_sb[:, dt_o:dt_o + 1],
                    xq4, OP.add, OP.add)

        # =============== P5: LN2 ===========================================
        xn2T = pers.tile([P, DT, R], F32R, tag="xnq_xn2")
        ln_T(lambda dt_i, ch: x1T[:, dt_i, ts(ch, 512)], R, g2,
             lambda dt_i, ch: xn2T[:, dt_i, ts(ch, 512)], stats_f32r=False)

        # =============== P6: fc1 + gelu ====================================
        hg = pers.tile([P, DFF // P, R], BF16, tag="xnT_hg")
        with (
            tc.tile_pool(name="w1p", bufs=2) as wp,
            tc.tile_pool(name="p6ps", bufs=4, space="PSUM") as pp,
        ):
            for mt in range(DFF // P):
                w1_sb = wp.tile([P, DT, P], F32R, tag="w1")
                nc.sync.dma_start(
                    w1_sb, w1_d[:, ts(mt, P)].rearrange("(t p) m -> p t m", p=P))
                ps = pp.tile([P, R], F32, tag="proj")
                for kt_i in range(DT):
                    nc.tensor.matmul(
                        ps, w1_sb[:, kt_i, :], xn2T[:, kt_i, :],
                        start=(kt_i == 0), stop=(kt_i == DT - 1))
                nc.scalar.activation(hg[:, mt, :], ps, AF.Gelu,
                                     bias=b1_sb[:, mt:mt + 1])

        # =============== P7: fc2 + bias + residual + store =================
        with (
            tc.tile_pool(name="w2p", bufs=2) as wp,
            tc.tile_pool(name="outp", bufs=2) as op_,
            tc.tile_pool(name="p7ps", bufs=4, space="PSUM") as pp,
        ):
            for dt_o in range(DT):
                w2_sb = wp.tile([P, DFF // P, P], BF16, tag="w2")
                nc.sync.dma_start(
                    w2_sb, w2_d[:, ts(dt_o, P)].rearrange("(t p) m -> p t m", p=P))
                ps = pp.tile([P, R], F32, tag="proj")
                for kt_i in range(DFF // P):
                    nc.tensor.matmul(
                        ps, w2_sb[:, kt_i, :], hg[:, kt_i, :],
                        start=(kt_i == 0), stop=(kt_i == DFF // P - 1))
                out_sb = op_.tile([P, R], F32, tag="out")
                nc.vector.scalar_tensor_tensor(
                    out_sb, ps, b2_sb[:, dt_o:dt_o + 1], x1T[:, dt_o, :],
                    OP.add, OP.add)
                nc.sync.dma_start(out_r[:, dt_o, :], out_sb)

    nc.compile()
    return nc


def _prep_in_maps(inputs):
    bf = ml_dtypes.bfloat16
    x = np.asarray(inputs["x"], np.float32)
    shared = {
        "wq": np.asarray(inputs["Wq"], np.float32).astype(bf),
        "wk": np.asarray(inputs["Wk"], np.float32).astype(bf),
        "wv": np.asarray(inputs["Wv"], np.float32).astype(bf),
        "wo": np.asarray(inputs["Wo"], np.float32).astype(bf),
        "w1": np.asarray(inputs["W1"], np.float32),
        "w2": np.asarray(inputs["W2"], np.float32).astype(bf),
        "bq": np.asarray(inputs["bq"], np.float32),
        "bk": np.asarray(inputs["bk"], np.float32),
        "bv": np.asarray(inputs["bv"], np.float32),
        "bo": np.asarray(inputs["bo"], np.float32),
        "b1": np.asarray(inputs["b1"], np.float32),
        "b2": np.asarray(inputs["b2"], np.float32),
        "ln1gb": np.stack([np.asarray(inputs["ln1_g"], np.float32),
                           np.asarray(inputs["ln1_b"], np.float32)]),
        "ln2gb": np.stack([np.asarray(inputs["ln2_g"], np.float32),
                           np.asarray(inputs["ln2_b"], np.float32)]),
    }
    in_maps = []
    for c in range(N_CORES):
        b, i = divmod(c, 4)
        lo = slice(QW * i, QW * i + QW)
        hi = slice(QW * (7 - i), QW * (8 - i))
        xb = x[b]                                    # [T, D]
        xq = np.concatenate([xb[lo], xb[hi]], 0)     # [R, D] (lo | hi)
        rows_lo = np.arange(QW * i, QW * i + QW)
        rows_hi = np.arange(QW * (7 - i), QW * (8 - i))
        kA = np.arange(T)[:, None]
        kB = np.arange(EB_KT * P)[:, None]
        maskA = np.where(kA <= rows_hi[None, :], np.float32(0), NEG).astype(bf)
        maskB = np.where(kB <= rows_lo[None, :], np.float32(0), NEG).astype(bf)
        in_maps.append({
            **shared,
            "xT": np.ascontiguousarray(xb.T),
            "xqT": np.ascontiguousarray(xq.T),
            "maskA": maskA,
            "maskB": maskB,
        })
    return in_maps


_PROGRAM = None


def _get_program():
    global _PROGRAM
    if _PROGRAM is None:
        _PROGRAM = _build_program()
    return _PROGRAM


def _assemble(results):
    out = np.empty((B, T, D), np.float32)
    for c in range(N_CORES):
        b, i = divmod(c, 4)
        oT = np.asarray(results[c]["outT"], np.float32)   # [D, R]
        out[b, QW * i:QW * i + QW] = oT[:, :QW].T
        out[b, QW * (7 - i):QW * (8 - i)] = oT[:, QW:].T
    return out


def kernel(**inputs):
    from concourse.bass_utils import run_bass_kernel_spmd
    nc = _get_program()
    in_maps = _prep_in_maps(inputs)
    res = run_bass_kernel_spmd(nc, in_maps, list(range(N_CORES)))
    return _assemble(res.results)


# revision 18
# speedup vs baseline: 1.1086x; 1.1086x over previous
"""Trainium2 Bass kernel: dense transformer block (LN -> causal MHA -> res -> LN -> MLP -> res).

Sharding: zero-communication query-parallel over 8 cores. Core c (b = c//4,
i = c%4) handles batch b's query chunks [256i, 256i+256) ("lo") and
[256(7-i), 256(8-i)) ("hi"). Every core computes K/V for its full batch
locally. Causality is exploited uniformly: the lo slot attends over keys
[0, 1024), the hi slot over [0, 2048), with host-supplied additive masks
making the program identical on all cores (SPMD requirement).

All activations are kept transposed ([d, tokens]) so the contraction dim
lands on SBUF partitions; the host transposes x on the way in and the
output on the way out. LayerNorm statistics are computed with ones-vector
matmuls on the PE; per-token scale/shift are broadcast across partitions
with rank-1/rank-2 matmuls. Softmax needs no max-subtraction (scores are
bounded: inputs are unit-scale, weights 0.02-scale), and the denominator
comes from an appended ones-column on V.
"""

import numpy as np
import ml_dtypes
from contextlib import ExitStack

import concourse.bass as bass
import concourse.mybir as mybir
import concourse.tile as tile
from concourse import bacc
from concourse.bass import ts

F32 = mybir.dt.float32
F32R = mybir.dt.float32r
BF16 = mybir.dt.bfloat16
AF = mybir.ActivationFunctionType
OP = mybir.AluOpType

B, T, D = 2, 2048, 1024
H, HD = 16, 64
DFF = 4096
P = 128
DT = D // P            # 8 d-tiles
KT = T // P            # 16 key tiles
R = 512                # query rows per core
QW = 256               # queries per slot
EA_KT = 16             # hi slot key tiles (keys [0, 2048))
EB_KT = 8              # lo slot key tiles (keys [0, 1024))
NEG = np.float32(-1e30)
N_CORES = 8


def _build_program():
    nc = bacc.Bacc(None, target_bir_lowering=False, debug=True)

    xT_d = nc.dram_tensor("xT", [D, T], F32R, kind="ExternalInput")
    xqT_d = nc.dram_tensor("xqT", [D, R], F32R, kind="ExternalInput")
    wq_d = nc.dram_tensor("wq", [D, D], BF16, kind="ExternalInput")
    wk_d = nc.dram_tensor("wk", [D, D], BF16, kind="ExternalInput")
    wv_d = nc.dram_tensor("wv", [D, D], BF16, kind="ExternalInput")
    wo_d = nc.dram_tensor("wo", [D, D], BF16, kind="ExternalInput")
    w1_d = nc.dram_tensor("w1", [D, DFF], BF16, kind="ExternalInput")
    w2_d = nc.dram_tensor("w2", [DFF, D], BF16, kind="ExternalInput")
    bq_d = nc.dram_tensor("bq", [D], F32, kind="ExternalInput")
    bk_d = nc.dram_tensor("bk", [D], F32, kind="ExternalInput")
    bv_d = nc.dram_tensor("bv", [D], F32R, kind="ExternalInput")
    bo_d = nc.dram_tensor("bo", [D], F32, kind="ExternalInput")
    b1_d = nc.dram_tensor("b1", [DFF], F32, kind="ExternalInput")
    b2_d = nc.dram_tensor("b2", [D], F32, kind="ExternalInput")
    g1_d = nc.dram_tensor("ln1gb", [2, D], F32R, kind="ExternalInput")
    g2_d = nc.dram_tensor("ln2gb", [2, D], F32R, kind="ExternalInput")
    mAB_d = nc.dram_tensor("maskAB", [EB_KT * P, R], BF16, kind="ExternalInput")
    mAhi_d = nc.dram_tensor("maskAhi", [EB_KT * P, QW], BF16, kind="ExternalInput")
    out_d = nc.dram_tensor("outT", [D, R], F32, kind="ExternalOutput")

    xT_r = xT_d[:, :].rearrange("(t p) n -> p t n", p=P)
    out_r = out_d[:, :].rearrange("(t p) n -> p t n", p=P)

    with tile.TileContext(nc) as tc, ExitStack() as ctx:
        ctx.enter_context(nc.allow_low_precision(
            reason="float32r outputs feed fp32r matmuls by design"))
        pers = ctx.enter_context(tc.tile_pool(name="pers", bufs=1))

        # ---- constants / small persistent tiles
        ones128 = pers.tile([P, 1], F32R)
        nc.vector.memset(ones128.bitcast(F32), 1.0)
        ones_row = pers.tile([1, P], F32R)
        nc.vector.memset(ones_row.bitcast(F32), 1.0)
        eps1 = pers.tile([1, 1], F32)
        nc.vector.memset(eps1, 1e-5)

        def load_stack(t0, tag):
            st = pers.tile([2, D], F32R, tag=tag)
            nc.sync.dma_start(st, t0[:, :])
            return st

        g1 = load_stack(g1_d, 'g1')
        g2 = load_stack(g2_d, 'g2')

        def load_bias_cols(src, ntiles, tag):
            t = pers.tile([P, ntiles], F32, tag=tag)
            nc.sync.dma_start(t, src[:].rearrange("(t p) -> p t", p=P))
            return t

        bo_sb = load_bias_cols(bo_d, DT, 'bo')
        bq_sb = load_bias_cols(bq_d, DT, 'bq')
        bk_sb = load_bias_cols(bk_d, DT, 'bk')
        b2_sb = load_bias_cols(b2_d, DT, 'b2')
        b1_sb = load_bias_cols(b1_d, DFF // P, 'b1')
        bv_row = pers.tile([1, D], F32R)
        nc.sync.dma_start(bv_row, bv_d[:].rearrange("(a d) -> a d", a=1))

        # ---- persistent activations (allocated up front, stack base)
        xnT = pers.tile([P, DT, T], BF16, tag="xnT_hg")       # 32KB/p, reused as hg
        xnqT = pers.tile([P, DT, R], BF16, tag="xnq_xn2")      # 16KB/p (shared w/ xn2T f32)
        KTs = pers.tile([P, DT, T], BF16)                      # 32KB/p
        Vs = pers.tile([P, KT, H * (HD + 1)], BF16)            # 32.5KB/p
        qTs = pers.tile([P, DT, R], BF16)                      # 8KB/p
        YTs = pers.tile([P, DT, R], BF16)                      # 8KB/p
        x1T = pers.tile([P, DT, R], F32)                       # 16KB/p

        # =============== transposed layernorm helper =================
        def ln_T(get_src, ncols, gstack, write_dst, stats_f32r=True):
            nch = ncols // 512
            with (
                tc.tile_pool(name="lnw", bufs=3) as lw,
                tc.tile_pool(name="lns", bufs=1) as lsm,
                tc.tile_pool(name="lnpa", bufs=2, space="PSUM") as pa,
                tc.tile_pool(name="lnpb", bufs=2, space="PSUM") as pb,
            ):
                for ch in range(nch):
                    sum_ps = pa.tile([1, 512], F32, tag="lnsum")
                    ssq_ps = pa.tile([1, 512], F32, tag="lnssq")
                    sdt = F32R if stats_f32r else F32
                    one_l = ones128 if stats_f32r else ones128.bitcast(F32)
                    for dt_i in range(DT):
                        xs = get_src(dt_i, ch)
                        sq = lw.tile([P, 512], sdt, tag="lnsq")
                        nc.vector.tensor_mul(sq, xs, xs)
                        nc.tensor.matmul(
                            sum_ps, one_l, xs,
                            start=(dt_i == 0), stop=(dt_i == DT - 1))
                        nc.tensor.matmul(
                            ssq_ps, one_l, sq[:, :],
                            start=(dt_i == 0), stop=(dt_i == DT - 1))
                    mu = lsm.tile([1, 512], F32, tag="mu")
                    nc.vector.tensor_scalar_mul(mu, sum_ps, 1.0 / D)
                    msq = lsm.tile([1, 512], F32, tag="msq")
                    nc.vector.tensor_scalar_mul(msq, ssq_ps, 1.0 / D)
                    musq = lsm.tile([1, 512], F32, tag="musq")
                    nc.vector.tensor_mul(musq, mu, mu)
                    nc.vector.tensor_sub(msq, msq, musq)
                    sd = lsm.tile([1, 512], F32, tag="sd")
                    nc.scalar.activation(sd, msq, AF.Sqrt, bias=eps1)
                    rsd = lsm.tile([1, 512], F32R, tag="rstd")
                    nc.vector.reciprocal(rsd, sd)
                    r2 = lsm.tile([2, 512], F32R, tag="r2")
                    nc.vector.memset(r2.bitcast(F32), 1.0)
                    nc.vector.scalar_tensor_tensor(
                        r2[0:1, :], mu, -1.0, rsd, OP.mult, OP.mult)
                    for dt_i in range(DT):
                        b1p = pb.tile([P, 512], F32, tag="lnb1")
                        nc.tensor.matmul(
                            b1p, gstack[0:1, ts(dt_i, P)],
                            rsd, start=True, stop=True)
                        b2p = pb.tile([P, 512], F32, tag="lnb2")
                        nc.tensor.matmul(
                            b2p, gstack[0:2, ts(dt_i, P)],
                            r2[:, :], start=True, stop=True)
                        xs = get_src(dt_i, ch)
                        t = lw.tile([P, 512], F32, tag="lnt")
                        nc.vector.tensor_mul(t, xs, b1p)
                        nc.vector.tensor_add(write_dst(dt_i, ch), t, b2p)

        # =============== P1: LN1 over full batch (streamed) and queries ====
        with tc.tile_pool(name="xstream", bufs=3) as xsp:
            def src_xT(dt_i, ch):
                t = xsp.tile([P, 512], F32R, tag="xs")
                nc.sync.dma_start(t, xT_r[:, dt_i, ts(ch, 512)])
                return t

            ln_T(src_xT, T, g1, lambda dt_i, ch: xnT[:, dt_i, ts(ch, 512)])

        xqT_r = xqT_d[:, :].rearrange("(t p) n -> p t n", p=P)
        with tc.tile_pool(name="xqstream", bufs=3) as xqp:
            def src_xq(dt_i, ch):
                t = xqp.tile([P, 512], F32R, tag="xqs")
                nc.sync.dma_start(t, xqT_r[:, dt_i, :])
                return t

            ln_T(src_xq, R, g1,
                 lambda dt_i, ch: xnqT[:, dt_i, ts(ch, 512)])

        # =============== P2: projections ====================================
        with (
            tc.tile_pool(name="wstream", bufs=2) as wp,
            tc.tile_pool(name="p2ps", bufs=4, space="PSUM") as pp,
        ):
            # Q^T = Wq^T @ xnq^T (+bq)
            wq_sb = wp.tile([P, DT, D], BF16, tag="w")
            nc.sync.dma_start(wq_sb, wq_d[:, :].rearrange("(t p) m -> p t m", p=P))
            for dt_o in range(DT):
                ps = pp.tile([P, R], F32, tag="proj")
                for kt_i in range(DT):
                    nc.tensor.matmul(
                        ps, wq_sb[:, kt_i, ts(dt_o, P)], xnqT[:, kt_i, :],
                        start=(kt_i == 0), stop=(kt_i == DT - 1))
                nc.vector.tensor_scalar_add(qTs[:, dt_o, :], ps,
                                            bq_sb[:, dt_o:dt_o + 1])
            # K^T = Wk^T @ xn^T (+bk)
            wk_sb = wp.tile([P, DT, D], BF16, tag="w")
            nc.sync.dma_start(wk_sb, wk_d[:, :].rearrange("(t p) m -> p t m", p=P))
            for dt_o in range(DT):
                for chn in range(T // 512):
                    ps = pp.tile([P, 512], F32, tag="proj")
                    for kt_i in range(DT):
                        nc.tensor.matmul(
                            ps, wk_sb[:, kt_i, ts(dt_o, P)],
                            xnT[:, kt_i, ts(chn, 512)],
                            start=(kt_i == 0), stop=(kt_i == DT - 1))
                    nc.vector.tensor_scalar_add(KTs[:, dt_o, ts(chn, 512)], ps,
                                                bk_sb[:, dt_o:dt_o + 1])
            # bv broadcast
            bvb = wp.tile([P, D], BF16, tag="bvb")
            for chn in range(2):
                ps = pp.tile([P, 512], F32, tag="proj")
                nc.tensor.matmul(ps, ones_row,
                                 bv_row[0:1, ts(chn, 512)],
                                 start=True, stop=True)
                nc.vector.tensor_copy(bvb[:, ts(chn, 512)], ps)
            # V natural = xn @ Wv (+bv), per-head interleave with ones column
            for vch in range(2):
                wv_sb = wp.tile([P, DT, 512], BF16, tag="w")
                nc.sync.dma_start(
                    wv_sb,
                    wv_d[:, ts(vch, 512)].rearrange("(t p) m -> p t m", p=P))
                for kto in range(KT):
                    ps = pp.tile([P, 512], F32, tag="proj")
                    for kt_i in range(DT):
                        nc.tensor.matmul(
                            ps, xnT[:, kt_i, ts(kto, P)], wv_sb[:, kt_i, :],
                            start=(kt_i == 0), stop=(kt_i == DT - 1))
                    dst = Vs[:, kto, :].rearrange(
                        "p (h c) -> p h c", c=HD + 1)[:, 8 * vch:8 * vch + 8, 0:HD]
                    nc.vector.tensor_add(
                        dst, ps.rearrange("p (h c) -> p h c", c=HD),
                        bvb[:, ts(vch, 512)].rearrange("p (h c) -> p h c", c=HD))
            ones_col = Vs[:, :, :].rearrange("p t (h c) -> p t h c", c=HD + 1)[:, :, :, HD:HD + 1]
            nc.vector.memset(ones_col, 1.0)

        # =============== P3: attention + P4: output proj ====================
        with (
            tc.tile_pool(name="maskp", bufs=1) as mp,
            tc.tile_pool(name="attw", bufs=4) as aw,
            tc.tile_pool(name="attw2", bufs=2) as aw2,
            tc.tile_pool(name="atts", bufs=2) as asml,
            tc.tile_pool(name="wop", bufs=1) as wop,
            tc.tile_pool(name="ps3s", bufs=3, space="PSUM") as ps3s,
            tc.tile_pool(name="ps3y", bufs=2, space="PSUM") as ps3y,
            tc.tile_pool(name="ps3b", bufs=1, space="PSUM") as ps3b,
        ):
            mAB_sb = mp.tile([P, EB_KT, R], BF16)
            nc.sync.dma_start(mAB_sb, mAB_d[:, :].rearrange("(t p) q -> p t q", p=P))
            mAhi_sb = mp.tile([P, EB_KT, QW], BF16)
            nc.sync.dma_start(mAhi_sb, mAhi_d[:, :].rearrange("(t p) q -> p t q", p=P))
            for h in range(H):
                base = (h % 2) * 64
                dt_h = h // 2
                vsl = slice(h * (HD + 1), (h + 1) * (HD + 1))
                y_ps = ps3y.tile([HD + 1, R], F32, tag="ypsum")
                # kt < 8: both slots at once (N=512)
                for kt_i in range(EB_KT):
                    s_ps = ps3s.tile([P, R], F32, tag="spsum")
                    nc.tensor.matmul(
                        s_ps, KTs[base:base + 64, dt_h, ts(kt_i, P)],
                        qTs[base:base + 64, dt_h, :],
                        start=True, stop=True)
                    p32 = aw.tile([P, R], F32, tag="p32")
                    nc.vector.tensor_add(p32, s_ps, mAB_sb[:, kt_i, :])
                    pbf = aw.tile([P, R], BF16, tag="pbf")
                    nc.scalar.activation(pbf, p32, AF.Exp, scale=0.125)
                    nc.tensor.matmul(
                        y_ps, Vs[:, kt_i, vsl], pbf,
                        start=(kt_i == 0), stop=False, skip_group_check=True)
                # kt >= 8: hi slot only (N=256)
                for kt_i in range(EB_KT, EA_KT):
                    s_ps = ps3s.tile([P, R], F32, tag="spsum")
                    nc.tensor.matmul(
                        s_ps[:, 0:QW], KTs[base:base + 64, dt_h, ts(kt_i, P)],
                        qTs[base:base + 64, dt_h, QW:R],
                        start=True, stop=True)
                    p32 = aw.tile([P, R], F32, tag="p32")
                    nc.vector.tensor_add(p32[:, 0:QW], s_ps[:, 0:QW],
                                         mAhi_sb[:, kt_i - EB_KT, :])
                    pbf = aw.tile([P, R], BF16, tag="pbf")
                    nc.scalar.activation(pbf[:, 0:QW], p32[:, 0:QW], AF.Exp,
                                         scale=0.125)
                    nc.tensor.matmul(
                        y_ps[:, QW:R], Vs[:, kt_i, vsl], pbf[:, 0:QW],
                        start=False, stop=(kt_i == EA_KT - 1),
                        skip_group_check=True)
                rec = asml.tile([1, R], F32R, tag="rec")
                nc.vector.reciprocal(rec, y_ps[HD:HD + 1, :])
                b_ps = ps3b.tile([64, R], F32, tag="bps")
                nc.tensor.matmul(b_ps, ones_row[0:1, 0:64],
                                 rec, start=True, stop=True)
                b_sb = aw2.tile([64, R], F32, tag="bsb")
                nc.vector.tensor_copy(b_sb, b_ps)
                nc.vector.tensor_mul(
                    YTs[base:base + 64, dt_h, :], y_ps[0:HD, :], b_sb)

            # output projection + residual
            wo_sb = wop.tile([P, DT, D], BF16)
            nc.sync.dma_start(wo_sb, wo_d[:, :].rearrange("(t p) m -> p t m", p=P))
            for dt_o in range(DT):
                ps = ps3s.tile([P, R], F32, tag="spsum")
                for kt_i in range(DT):
                    nc.tensor.matmul(
                        ps, wo_sb[:, kt_i, ts(dt_o, P)], YTs[:, kt_i, :],
                        start=(kt_i == 0), stop=(kt_i == DT - 1))
                xq4 = aw2.tile([P, R], F32R, tag="xq4")
                nc.sync.dma_start(xq4, xqT_r[:, dt_o, :])
                nc.vector.scalar_tensor_tensor(
                    x1T[:, dt_o, :], ps, bo_sb[:, dt_o:dt_o + 1],
                    xq4, OP.add, OP.add)

        # =============== P5: LN2 ===========================================
        xn2T = pers.tile([P, DT, R], BF16, tag="xnq_xn2")
        ln_T(lambda dt_i, ch: x1T[:, dt_i, ts(ch, 512)], R, g2,
             lambda dt_i, ch: xn2T[:, dt_i, ts(ch, 512)], stats_f32r=False)

        # =============== P6: fc1 + gelu ====================================
        hg = pers.tile([P, DFF // P, R], BF16, tag="xnT_hg")
        with (
            tc.tile_pool(name="w1p", bufs=2) as wp,
            tc.tile_pool(name="p6ps", bufs=4, space="PSUM") as pp,
        ):
            for mt in range(DFF // P):
                w1_sb = wp.tile([P, DT, P], BF16, tag="w1")
                nc.sync.dma_start(
                    w1_sb, w1_d[:, ts(mt, P)].rearrange("(t p) m -> p t m", p=P))
                ps = pp.tile([P, R], F32, tag="proj")
                for kt_i in range(DT):
                    nc.tensor.matmul(
                        ps, w1_sb[:, kt_i, :], xn2T[:, kt_i, :],
                        start=(kt_i == 0), stop=(kt_i == DT - 1))
                nc.scalar.activation(hg[:, mt, :], ps, AF.Gelu,
                                     bias=b1_sb[:, mt:mt + 1])

        # =============== P7: fc2 + bias + residual + store =================
        with (
            tc.tile_pool(name="w2p", bufs=2) as wp,
            tc.tile_pool(name="outp", bufs=2) as op_,
            tc.tile_pool(name="p7ps", bufs=4, space="PSUM") as pp,
        ):
            for dt_o in range(DT):
                w2_sb = wp.tile([P, DFF // P, P], BF16, tag="w2")
                nc.sync.dma_start(
                    w2_sb, w2_d[:, ts(dt_o, P)].rearrange("(t p) m -> p t m", p=P))
                ps = pp.tile([P, R], F32, tag="proj")
                for kt_i in range(DFF // P):
                    nc.tensor.matmul(
                        ps, w2_sb[:, kt_i, :], hg[:, kt_i, :],
                        start=(kt_i == 0), stop=(kt_i == DFF // P - 1))
                out_sb = op_.tile([P, R], F32, tag="out")
                nc.vector.scalar_tensor_tensor(
                    out_sb, ps, b2_sb[:, dt_o:dt_o + 1], x1T[:, dt_o, :],
                    OP.add, OP.add)
                nc.sync.dma_start(out_r[:, dt_o, :], out_sb)

    nc.compile()
    return nc


def _prep_in_maps(inputs):
    bf = ml_dtypes.bfloat16
    x = np.asarray(inputs["x"], np.float32)
    shared = {
        "wq": np.asarray(inputs["Wq"], np.float32).astype(bf),
        "wk": np.asarray(inputs["Wk"], np.float32).astype(bf),
        "wv": np.asarray(inputs["Wv"], np.float32).astype(bf),
        "wo": np.asarray(inputs["Wo"], np.float32).astype(bf),
        "w1": np.asarray(inputs["W1"], np.float32).astype(bf),
        "w2": np.asarray(inputs["W2"], np.float32).astype(bf),
        "bq": np.asarray(inputs["bq"], np.float32),
        "bk": np.asarray(inputs["bk"], np.float32),
        "bv": np.asarray(inputs["bv"], np.float32),
        "bo": np.asarray(inputs["bo"], np.float32),
        "b1": np.asarray(inputs["b1"], np.float32),
        "b2": np.asarray(inputs["b2"], np.float32),
        "ln1gb": np.stack([np.asarray(inputs["ln1_g"], np.float32),
                           np.asarray(inputs["ln1_b"], np.float32)]),
        "ln2gb": np.stack([np.asarray(inputs["ln2_g"], np.float32),
                           np.asarray(inputs["ln2_b"], np.float32)]),
    }
    in_maps = []
    for c in range(N_CORES):
        b, i = divmod(c, 4)
        lo = slice(QW * i, QW * i + QW)
        hi = slice(QW * (7 - i), QW * (8 - i))
        xb = x[b]                                    # [T, D]
        xq = np.concatenate([xb[lo], xb[hi]], 0)     # [R, D] (lo | hi)
        rows_lo = np.arange(QW * i, QW * i + QW)
        rows_hi = np.arange(QW * (7 - i), QW * (8 - i))
        kA = np.arange(T)[:, None]
        kB = np.arange(EB_KT * P)[:, None]
        maskA = np.where(kA <= rows_hi[None, :], np.float32(0), NEG).astype(bf)
        maskB = np.where(kB <= rows_lo[None, :], np.float32(0), NEG).astype(bf)
        maskAB = np.concatenate([maskB, maskA[:EB_KT * P]], axis=1)  # [1024, 512]
        in_maps.append({
            **shared,
            "xT": np.ascontiguousarray(xb.T),
            "xqT": np.ascontiguousarray(xq.T),
            "maskAB": maskAB,
            "maskAhi": np.ascontiguousarray(maskA[EB_KT * P:]),
        })
    return in_maps


_PROGRAM = None


def _get_program():
    global _PROGRAM
    if _PROGRAM is None:
        _PROGRAM = _build_program()
    return _PROGRAM


def _assemble(results):
    out = np.empty((B, T, D), np.float32)
    for c in range(N_CORES):
        b, i = divmod(c, 4)
        oT = np.asarray(results[c]["outT"], np.float32)   # [D, R]
        out[b, QW * i:QW * i + QW] = oT[:, :QW].T
        out[b, QW * (7 - i):QW * (8 - i)] = oT[:, QW:].T
    return out


def kernel(**inputs):
    from concourse.bass_utils import run_bass_kernel_spmd
    nc = _get_program()
    in_maps = _prep_in_maps(inputs)
    res = run_bass_kernel_spmd(nc, in_maps, list(range(N_CORES)))
    return _assemble(res.results)


# revision 19
# speedup vs baseline: 1.1969x; 1.0797x over previous
"""Trainium2 Bass kernel: dense transformer block (LN -> causal MHA -> res -> LN -> MLP -> res).

Sharding: zero-communication query-parallel over 8 cores. Core c (b = c//4,
i = c%4) handles batch b's query chunks [256i, 256i+256) ("lo") and
[256(7-i), 256(8-i)) ("hi"). Every core computes K/V for its full batch
locally. Causality is exploited uniformly: the lo slot attends over keys
[0, 1024), the hi slot over [0, 2048), with host-supplied additive masks
making the program identical on all cores (SPMD requirement).

All activations are kept transposed ([d, tokens]) so contraction dims land
on SBUF partitions; the host transposes x on the way in and the output on
the way out. LayerNorm statistics use ones-vector matmuls on the PE;
per-token scale/shift broadcast across partitions via rank-1/rank-2
matmuls. Softmax needs no max-subtraction (scores are bounded), and the
denominator comes from an appended ones-column on V.
"""

import numpy as np
import ml_dtypes
from contextlib import ExitStack

import concourse.bass as bass
import concourse.mybir as mybir
import concourse.tile as tile
from concourse import bacc
from concourse.bass import ts

F32 = mybir.dt.float32
F32R = mybir.dt.float32r
BF16 = mybir.dt.bfloat16
AF = mybir.ActivationFunctionType
OP = mybir.AluOpType

B, T, D = 2, 2048, 1024
H, HD = 16, 64
DFF = 4096
P = 128
DT = D // P            # 8 d-tiles
MT = DFF // P          # 32 dff-tiles
KT = T // P            # 16 key tiles
R = 512                # query rows per core
QW = 256               # queries per slot
EA_KT = 16             # hi slot key tiles (keys [0, 2048))
EB_KT = 8              # lo slot key tiles (keys [0, 1024))
NEG = np.float32(-1e30)
N_CORES = 8
PIPE = 3               # attention software-pipeline depth


def _build_program():
    nc = bacc.Bacc(None, target_bir_lowering=False, debug=True)

    xT_d = nc.dram_tensor("xT", [D, T], F32R, kind="ExternalInput")
    xqT_d = nc.dram_tensor("xqT", [D, R], F32R, kind="ExternalInput")
    wq_d = nc.dram_tensor("wq", [D, D], BF16, kind="ExternalInput")
    wk_d = nc.dram_tensor("wk", [D, D], BF16, kind="ExternalInput")
    wv_d = nc.dram_tensor("wv", [D, D], BF16, kind="ExternalInput")
    wo_d = nc.dram_tensor("wo", [D, D], BF16, kind="ExternalInput")
    w1_d = nc.dram_tensor("w1p", [MT, P, DT, P], BF16, kind="ExternalInput")
    w2_d = nc.dram_tensor("w2p", [DT, P, MT, P], BF16, kind="ExternalInput")
    bq_d = nc.dram_tensor("bq", [D], F32, kind="ExternalInput")
    bk_d = nc.dram_tensor("bk", [D], F32, kind="ExternalInput")
    bv_d = nc.dram_tensor("bv", [D], F32R, kind="ExternalInput")
    bo_d = nc.dram_tensor("bo", [D], F32, kind="ExternalInput")
    b1_d = nc.dram_tensor("b1", [DFF], F32, kind="ExternalInput")
    b2_d = nc.dram_tensor("b2", [D], F32, kind="ExternalInput")
    g1_d = nc.dram_tensor("ln1gb", [2, D], F32R, kind="ExternalInput")
    g2_d = nc.dram_tensor("ln2gb", [2, D], F32R, kind="ExternalInput")
    mAB_d = nc.dram_tensor("maskAB", [EB_KT * P, R], BF16, kind="ExternalInput")
    mAhi_d = nc.dram_tensor("maskAhi", [EB_KT * P, QW], BF16, kind="ExternalInput")
    out_d = nc.dram_tensor("outT", [D, R], F32, kind="ExternalOutput")

    xT_r = xT_d[:, :].rearrange("(t p) n -> p t n", p=P)
    xqT_r = xqT_d[:, :].rearrange("(t p) n -> p t n", p=P)
    out_r = out_d[:, :].rearrange("(t p) n -> p t n", p=P)

    with tile.TileContext(nc) as tc, ExitStack() as ctx:
        ctx.enter_context(nc.allow_low_precision(
            reason="float32r outputs feed fp32r matmuls by design"))
        pers = ctx.enter_context(tc.tile_pool(name="pers", bufs=1))

        # ---- constants / small persistent tiles
        ones128 = pers.tile([P, 1], F32R)
        nc.vector.memset(ones128.bitcast(F32), 1.0)
        ones_row = pers.tile([1, P], F32R)
        nc.vector.memset(ones_row.bitcast(F32), 1.0)
        eps1 = pers.tile([1, 1], F32)
        nc.vector.memset(eps1, 1e-5)

        def load_stack(t0, tag):
            st = pers.tile([2, D], F32R, tag=tag)
            nc.sync.dma_start(st, t0[:, :])
            return st

        g1 = load_stack(g1_d, 'g1')
        g2 = load_stack(g2_d, 'g2')

        def load_bias_cols(src, ntiles, tag):
            t = pers.tile([P, ntiles], F32, tag=tag)
            nc.sync.dma_start(t, src[:].rearrange("(t p) -> p t", p=P))
            return t

        bo_sb = load_bias_cols(bo_d, DT, 'bo')
        bq_sb = load_bias_cols(bq_d, DT, 'bq')
        bk_sb = load_bias_cols(bk_d, DT, 'bk')
        b2_sb = load_bias_cols(b2_d, DT, 'b2')
        b1_sb = load_bias_cols(b1_d, MT, 'b1')
        bv_row = pers.tile([1, D], F32R)
        nc.sync.dma_start(bv_row, bv_d[:].rearrange("(a d) -> a d", a=1))

        # ---- persistent activations (allocated up front, stack base)
        xnT = pers.tile([P, DT, T], BF16, tag="xnT_hg")       # 32KB/p, reused as hg
        xnqT = pers.tile([P, DT, R], BF16, tag="xnq_xn2")      # 16KB/p (shared w/ xn2T)
        KTs = pers.tile([P, DT, T], BF16)                      # 32KB/p
        Vs = pers.tile([P, KT, H * (HD + 1)], BF16)            # 32.5KB/p
        qTs = pers.tile([P, DT, R], BF16)                      # 8KB/p
        YTs = pers.tile([P, DT, R], BF16)                      # 8KB/p
        x1T = pers.tile([P, DT, R], F32)                       # 16KB/p

        # =============== transposed layernorm helper =================
        # Emits, per 512-token chunk: PE stats (ones-matmul sums), scalar
        # math, rank-1/2 broadcast matmuls, and the normalize ops; then
        # calls post_chunk(ch) so downstream consumers of the chunk can be
        # interleaved for PE density.
        def ln_T(get_src, ncols, gstack, write_dst, stats_f32r=True,
                 post_chunk=None):
            nch = ncols // 512
            with (
                tc.tile_pool(name="lnw", bufs=3) as lw,
                tc.tile_pool(name="lns", bufs=1) as lsm,
                tc.tile_pool(name="lnpa", bufs=1, space="PSUM") as pa,
                tc.tile_pool(name="lnpb", bufs=2, space="PSUM") as pb,
            ):
                for ch in range(nch):
                    sum_ps = pa.tile([1, 512], F32, tag="lnsum")
                    ssq_ps = pa.tile([1, 512], F32, tag="lnssq")
                    sdt = F32R if stats_f32r else F32
                    one_l = ones128 if stats_f32r else ones128.bitcast(F32)
                    for dt_i in range(DT):
                        xs = get_src(dt_i, ch)
                        sq = lw.tile([P, 512], sdt, tag="lnsq")
                        nc.vector.tensor_mul(sq, xs, xs)
                        nc.tensor.matmul(
                            sum_ps, one_l, xs,
                            start=(dt_i == 0), stop=(dt_i == DT - 1))
                        nc.tensor.matmul(
                            ssq_ps, one_l, sq[:, :],
                            start=(dt_i == 0), stop=(dt_i == DT - 1))
                    mu = lsm.tile([1, 512], F32, tag="mu")
                    nc.vector.tensor_scalar_mul(mu, sum_ps, 1.0 / D)
                    msq = lsm.tile([1, 512], F32, tag="msq")
                    nc.vector.tensor_scalar_mul(msq, ssq_ps, 1.0 / D)
                    musq = lsm.tile([1, 512], F32, tag="musq")
                    nc.vector.tensor_mul(musq, mu, mu)
                    nc.vector.tensor_sub(msq, msq, musq)
                    sd = lsm.tile([1, 512], F32, tag="sd")
                    nc.scalar.activation(sd, msq, AF.Sqrt, bias=eps1)
                    rsd = lsm.tile([1, 512], F32R, tag="rstd")
                    nc.vector.reciprocal(rsd, sd)
                    r2 = lsm.tile([2, 512], F32R, tag="r2")
                    nc.vector.memset(r2.bitcast(F32), 1.0)
                    nc.vector.scalar_tensor_tensor(
                        r2[0:1, :], mu, -1.0, rsd, OP.mult, OP.mult)
                    for dt_i in range(DT):
                        b1p = pb.tile([P, 512], F32, tag="lnb1")
                        nc.tensor.matmul(
                            b1p, gstack[0:1, ts(dt_i, P)],
                            rsd, start=True, stop=True)
                        b2p = pb.tile([P, 512], F32, tag="lnb2")
                        nc.tensor.matmul(
                            b2p, gstack[0:2, ts(dt_i, P)],
                            r2[:, :], start=True, stop=True)
                        xs = get_src(dt_i, ch)
                        t = lw.tile([P, 512], F32, tag="lnt")
                        nc.vector.tensor_mul(t, xs, b1p)
                        nc.vector.tensor_add(write_dst(dt_i, ch), t, b2p)
                    if post_chunk is not None:
                        post_chunk(ch)

        # =============== P1+P2: LN1 (streamed) interleaved with K-proj,
        # then Q and V projections ==========================================
        with (
            tc.tile_pool(name="xstream", bufs=6) as xsp,
            tc.tile_pool(name="wstream", bufs=1) as wp,
            tc.tile_pool(name="p2ps", bufs=2, space="PSUM") as pp,
        ):
            wk_sb = wp.tile([P, DT, D], BF16, tag="w")
            nc.sync.dma_start(wk_sb, wk_d[:, :].rearrange("(t p) m -> p t m", p=P))

            def src_xT(dt_i, ch):
                t = xsp.tile([P, 512], F32R, tag="xs")
                nc.sync.dma_start(t, xT_r[:, dt_i, ts(ch, 512)])
                return t

            def k_proj_chunk(chn):
                for dt_o in range(DT):
                    ps = pp.tile([P, 512], F32, tag="proj")
                    for kt_i in range(DT):
                        nc.tensor.matmul(
                            ps, wk_sb[:, kt_i, ts(dt_o, P)],
                            xnT[:, kt_i, ts(chn, 512)],
                            start=(kt_i == 0), stop=(kt_i == DT - 1))
                    nc.vector.tensor_scalar_add(KTs[:, dt_o, ts(chn, 512)], ps,
                                                bk_sb[:, dt_o:dt_o + 1])

            ln_T(src_xT, T, g1, lambda dt_i, ch: xnT[:, dt_i, ts(ch, 512)],
                 post_chunk=k_proj_chunk)

            # LN1 on query rows
            def src_xq(dt_i, ch):
                t = xsp.tile([P, 512], F32R, tag="xs")
                nc.sync.dma_start(t, xqT_r[:, dt_i, :])
                return t

            ln_T(src_xq, R, g1,
                 lambda dt_i, ch: xnqT[:, dt_i, ts(ch, 512)])

            # Q^T = Wq^T @ xnq^T (+bq)
            wq_sb = wp.tile([P, DT, D], BF16, tag="w")
            nc.sync.dma_start(wq_sb, wq_d[:, :].rearrange("(t p) m -> p t m", p=P))
            for dt_o in range(DT):
                ps = pp.tile([P, R], F32, tag="proj")
                for kt_i in range(DT):
                    nc.tensor.matmul(
                        ps, wq_sb[:, kt_i, ts(dt_o, P)], xnqT[:, kt_i, :],
                        start=(kt_i == 0), stop=(kt_i == DT - 1))
                nc.vector.tensor_scalar_add(qTs[:, dt_o, :], ps,
                                            bq_sb[:, dt_o:dt_o + 1])

            # bv broadcast across partitions
            bvb = wp.tile([P, D], BF16, tag="bvb")
            for chn in range(2):
                ps = pp.tile([P, 512], F32, tag="proj")
                nc.tensor.matmul(ps, ones_row,
                                 bv_row[0:1, ts(chn, 512)],
                                 start=True, stop=True)
                nc.vector.tensor_copy(bvb[:, ts(chn, 512)], ps)

            # V natural = xn @ Wv (+bv), per-head interleave + ones column
            for vch in range(2):
                wv_sb = wp.tile([P, DT, 512], BF16, tag="w")
                nc.sync.dma_start(
                    wv_sb,
                    wv_d[:, ts(vch, 512)].rearrange("(t p) m -> p t m", p=P))
                for kto in range(KT):
                    ps = pp.tile([P, 512], F32, tag="proj")
                    for kt_i in range(DT):
                        nc.tensor.matmul(
                            ps, xnT[:, kt_i, ts(kto, P)], wv_sb[:, kt_i, :],
                            start=(kt_i == 0), stop=(kt_i == DT - 1))
                    dst = Vs[:, kto, :].rearrange(
                        "p (h c) -> p h c", c=HD + 1)[:, 8 * vch:8 * vch + 8, 0:HD]
                    nc.vector.tensor_add(
                        dst, ps.rearrange("p (h c) -> p h c", c=HD),
                        bvb[:, ts(vch, 512)].rearrange("p (h c) -> p h c", c=HD))
            ones_col = Vs[:, :, :].rearrange(
                "p t (h c) -> p t h c", c=HD + 1)[:, :, :, HD:HD + 1]
            nc.vector.memset(ones_col, 1.0)

        # =============== P3: attention (sw-pipelined) + output proj ========
        with (
            tc.tile_pool(name="maskp", bufs=1) as mp,
            tc.tile_pool(name="attw", bufs=PIPE + 1) as aw,
            tc.tile_pool(name="attw2", bufs=2) as aw2,
            tc.tile_pool(name="atts", bufs=2) as asml,
            tc.tile_pool(name="wop", bufs=1) as wop,
            tc.tile_pool(name="ps3s", bufs=PIPE + 1, space="PSUM") as ps3s,
            tc.tile_pool(name="ps3y", bufs=2, space="PSUM") as ps3y,
            tc.tile_pool(name="ps3b", bufs=1, space="PSUM") as ps3b,
        ):
            mAB_sb = mp.tile([P, EB_KT, R], BF16)
            nc.sync.dma_start(mAB_sb, mAB_d[:, :].rearrange("(t p) q -> p t q", p=P))
            mAhi_sb = mp.tile([P, EB_KT, QW], BF16)
            nc.sync.dma_start(mAhi_sb, mAhi_d[:, :].rearrange("(t p) q -> p t q", p=P))
            for h in range(H):
                base = (h % 2) * 64
                dt_h = h // 2
                vsl = slice(h * (HD + 1), (h + 1) * (HD + 1))
                y_ps = ps3y.tile([HD + 1, R], F32, tag="ypsum")
                pbfs = {}

                def front(kt_i, base=base, dt_h=dt_h, pbfs=pbfs):
                    full = kt_i < EB_KT
                    qsl = slice(0, R) if full else slice(QW, R)
                    w = R if full else QW
                    s_ps = ps3s.tile([P, R], F32, tag="spsum")
                    nc.tensor.matmul(
                        s_ps[:, 0:w], KTs[base:base + 64, dt_h, ts(kt_i, P)],
                        qTs[base:base + 64, dt_h, qsl],
                        start=True, stop=True)
                    p32 = aw.tile([P, R], F32, tag="p32")
                    m_ap = (mAB_sb[:, kt_i, :] if full
                            else mAhi_sb[:, kt_i - EB_KT, :])
                    nc.vector.tensor_add(p32[:, 0:w], s_ps[:, 0:w], m_ap)
                    pbf = aw.tile([P, R], BF16, tag="pbf")
                    nc.scalar.activation(pbf[:, 0:w], p32[:, 0:w], AF.Exp,
                                         scale=0.125)
                    pbfs[kt_i] = pbf

                def av(kt_i, y_ps=y_ps, vsl=vsl, pbfs=pbfs):
                    full = kt_i < EB_KT
                    osl = slice(0, R) if full else slice(QW, R)
                    w = R if full else QW
                    nc.tensor.matmul(
                        y_ps[:, osl], Vs[:, kt_i, vsl], pbfs.pop(kt_i)[:, 0:w],
                        start=(kt_i == 0), stop=(kt_i == EA_KT - 1),
                        skip_group_check=True)

                for kt_i in range(PIPE):
                    front(kt_i)
                for kt_i in range(EA_KT):
                    if kt_i + PIPE < EA_KT:
                        front(kt_i + PIPE)
                    av(kt_i)

                rec = asml.tile([1, R], F32R, tag="rec")
                nc.vector.reciprocal(rec, y_ps[HD:HD + 1, :])
                b_ps = ps3b.tile([64, R], F32, tag="bps")
                nc.tensor.matmul(b_ps, ones_row[0:1, 0:64],
                                 rec, start=True, stop=True)
                b_sb = aw2.tile([64, R], F32, tag="bsb")
                nc.vector.tensor_copy(b_sb, b_ps)
                nc.vector.tensor_mul(
                    YTs[base:base + 64, dt_h, :], y_ps[0:HD, :], b_sb)

            # output projection + residual
            wo_sb = wop.tile([P, DT, D], BF16)
            nc.sync.dma_start(wo_sb, wo_d[:, :].rearrange("(t p) m -> p t m", p=P))
            for dt_o in range(DT):
                ps = ps3s.tile([P, R], F32, tag="spsum")
                for kt_i in range(DT):
                    nc.tensor.matmul(
                        ps, wo_sb[:, kt_i, ts(dt_o, P)], YTs[:, kt_i, :],
                        start=(kt_i == 0), stop=(kt_i == DT - 1))
                xq4 = aw2.tile([P, R], F32R, tag="xq4")
                nc.sync.dma_start(xq4, xqT_r[:, dt_o, :])
                nc.vector.scalar_tensor_tensor(
                    x1T[:, dt_o, :], ps, bo_sb[:, dt_o:dt_o + 1],
                    xq4, OP.add, OP.add)

        # =============== P5: LN2 ===========================================
        xn2T = pers.tile([P, DT, R], BF16, tag="xnq_xn2")
        ln_T(lambda dt_i, ch: x1T[:, dt_i, ts(ch, 512)], R, g2,
             lambda dt_i, ch: xn2T[:, dt_i, ts(ch, 512)], stats_f32r=False)

        # =============== P6: fc1 + gelu ====================================
        hg = pers.tile([P, MT, R], BF16, tag="xnT_hg")
        with (
            tc.tile_pool(name="w1p", bufs=3) as wp1,
            tc.tile_pool(name="p6ps", bufs=4, space="PSUM") as pp6,
        ):
            for mt in range(MT):
                w1_sb = wp1.tile([P, DT, P], BF16, tag="w1")
                nc.sync.dma_start(w1_sb, w1_d[mt])
                ps = pp6.tile([P, R], F32, tag="proj")
                for kt_i in range(DT):
                    nc.tensor.matmul(
                        ps, w1_sb[:, kt_i, :], xn2T[:, kt_i, :],
                        start=(kt_i == 0), stop=(kt_i == DT - 1))
                nc.scalar.activation(hg[:, mt, :], ps, AF.Gelu,
                                     bias=b1_sb[:, mt:mt + 1])

        # =============== P7: fc2 + bias + residual + store =================
        with (
            tc.tile_pool(name="w2p", bufs=2) as wp2,
            tc.tile_pool(name="outp", bufs=2) as op_,
            tc.tile_pool(name="p7ps", bufs=4, space="PSUM") as pp7,
        ):
            for dt_o in range(DT):
                w2_sb = wp2.tile([P, MT, P], BF16, tag="w2")
                nc.sync.dma_start(w2_sb, w2_d[dt_o])
                ps = pp7.tile([P, R], F32, tag="proj")
                for kt_i in range(MT):
                    nc.tensor.matmul(
                        ps, w2_sb[:, kt_i, :], hg[:, kt_i, :],
                        start=(kt_i == 0), stop=(kt_i == MT - 1))
                out_sb = op_.tile([P, R], F32, tag="out")
                nc.vector.scalar_tensor_tensor(
                    out_sb, ps, b2_sb[:, dt_o:dt_o + 1], x1T[:, dt_o, :],
                    OP.add, OP.add)
                nc.sync.dma_start(out_r[:, dt_o, :], out_sb)

    nc.compile()
    return nc


def _prep_in_maps(inputs):
    bf = ml_dtypes.bfloat16
    x = np.asarray(inputs["x"], np.float32)
    w1 = np.asarray(inputs["W1"], np.float32).astype(bf)   # [D, DFF]
    w2 = np.asarray(inputs["W2"], np.float32).astype(bf)   # [DFF, D]
    # packed layouts for contiguous weight-tile DMA:
    # w1p[mt, p, kt, m] = W1[kt*128+p, mt*128+m]
    w1p = np.ascontiguousarray(
        w1.reshape(DT, P, MT, P).transpose(2, 1, 0, 3))
    # w2p[dt, p, kt, m] = W2[kt*128+p, dt*128+m]
    w2p = np.ascontiguousarray(
        w2.reshape(MT, P, DT, P).transpose(2, 1, 0, 3))
    shared = {
        "wq": np.asarray(inputs["Wq"], np.float32).astype(bf),
        "wk": np.asarray(inputs["Wk"], np.float32).astype(bf),
        "wv": np.asarray(inputs["Wv"], np.float32).astype(bf),
        "wo": np.asarray(inputs["Wo"], np.float32).astype(bf),
        "w1p": w1p,
        "w2p": w2p,
        "bq": np.asarray(inputs["bq"], np.float32),
        "bk": np.asarray(inputs["bk"], np.float32),
        "bv": np.asarray(inputs["bv"], np.float32),
        "bo": np.asarray(inputs["bo"], np.float32),
        "b1": np.asarray(inputs["b1"], np.float32),
        "b2": np.asarray(inputs["b2"], np.float32),
        "ln1gb": np.stack([np.asarray(inputs["ln1_g"], np.float32),
                           np.asarray(inputs["ln1_b"], np.float32)]),
        "ln2gb": np.stack([np.asarray(inputs["ln2_g"], np.float32),
                           np.asarray(inputs["ln2_b"], np.float32)]),
    }
    in_maps = []
    for c in range(N_CORES):
        b, i = divmod(c, 4)
        lo = slice(QW * i, QW * i + QW)
        hi = slice(QW * (7 - i), QW * (8 - i))
        xb = x[b]                                    # [T, D]
        xq = np.concatenate([xb[lo], xb[hi]], 0)     # [R, D] (lo | hi)
        rows_lo = np.arange(QW * i, QW * i + QW)
        rows_hi = np.arange(QW * (7 - i), QW * (8 - i))
        kA = np.arange(T)[:, None]
        kB = np.arange(EB_KT * P)[:, None]
        maskA = np.where(kA <= rows_hi[None, :], np.float32(0), NEG).astype(bf)
        maskB = np.where(kB <= rows_lo[None, :], np.float32(0), NEG).astype(bf)
        maskAB = np.concatenate([maskB, maskA[:EB_KT * P]], axis=1)  # [1024, 512]
        in_maps.append({
            **shared,
            "xT": np.ascontiguousarray(xb.T),
            "xqT": np.ascontiguousarray(xq.T),
            "maskAB": maskAB,
            "maskAhi": np.ascontiguousarray(maskA[EB_KT * P:]),
        })
    return in_maps


_PROGRAM = None


def _get_program():
    global _PROGRAM
    if _PROGRAM is None:
        _PROGRAM = _build_program()
    return _PROGRAM


def _assemble(results):
    out = np.empty((B, T, D), np.float32)
    for c in range(N_CORES):
        b, i = divmod(c, 4)
        oT = np.asarray(results[c]["outT"], np.float32)   # [D, R]
        out[b, QW * i:QW * i + QW] = oT[:, :QW].T
        out[b, QW * (7 - i):QW * (8 - i)] = oT[:, QW:].T
    return out


def kernel(**inputs):
    from concourse.bass_utils import run_bass_kernel_spmd
    nc = _get_program()
    in_maps = _prep_in_maps(inputs)
    res = run_bass_kernel_spmd(nc, in_maps, list(range(N_CORES)))
    return _assemble(res.results)


# revision 20
# speedup vs baseline: 1.1992x; 1.0019x over previous
"""Trainium2 Bass kernel: dense transformer block (LN -> causal MHA -> res -> LN -> MLP -> res).

Sharding: zero-communication query-parallel over 8 cores. Core c (b = c//4,
i = c%4) handles batch b's query chunks [256i, 256i+256) ("lo") and
[256(7-i), 256(8-i)) ("hi"). Every core computes K/V for its full batch
locally. Causality is exploited uniformly: the lo slot attends over keys
[0, 1024), the hi slot over [0, 2048), with host-supplied additive masks
making the program identical on all cores (SPMD requirement).

All activations are kept transposed ([d, tokens]) so contraction dims land
on SBUF partitions; the host transposes x on the way in and the output on
the way out. LayerNorm statistics use ones-vector matmuls on the PE;
per-token scale/shift broadcast across partitions via rank-1/rank-2
matmuls. Softmax needs no max-subtraction (scores are bounded), and the
denominator comes from an appended ones-column on V.
"""

import numpy as np
import ml_dtypes
from contextlib import ExitStack

import concourse.bass as bass
import concourse.mybir as mybir
import concourse.tile as tile
from concourse import bacc
from concourse.bass import ts

F32 = mybir.dt.float32
F32R = mybir.dt.float32r
BF16 = mybir.dt.bfloat16
AF = mybir.ActivationFunctionType
OP = mybir.AluOpType

B, T, D = 2, 2048, 1024
H, HD = 16, 64
DFF = 4096
P = 128
DT = D // P            # 8 d-tiles
MT = DFF // P          # 32 dff-tiles
KT = T // P            # 16 key tiles
R = 512                # query rows per core
QW = 256               # queries per slot
EA_KT = 16             # hi slot key tiles (keys [0, 2048))
EB_KT = 8              # lo slot key tiles (keys [0, 1024))
NEG = np.float32(-1e30)
N_CORES = 8
PIPE = 3               # attention software-pipeline depth


def _build_program():
    nc = bacc.Bacc(None, target_bir_lowering=False, debug=True)

    xT_d = nc.dram_tensor("xT", [D, T], F32R, kind="ExternalInput")
    xqT_d = nc.dram_tensor("xqT", [D, R], F32R, kind="ExternalInput")
    wq_d = nc.dram_tensor("wq", [D, D], BF16, kind="ExternalInput")
    wk_d = nc.dram_tensor("wk", [D, D], BF16, kind="ExternalInput")
    wv_d = nc.dram_tensor("wv", [D, D], BF16, kind="ExternalInput")
    wo_d = nc.dram_tensor("wo", [D, D], BF16, kind="ExternalInput")
    w1_d = nc.dram_tensor("w1p", [MT, P, DT, P], BF16, kind="ExternalInput")
    w2_d = nc.dram_tensor("w2p", [DT, P, MT, P], BF16, kind="ExternalInput")
    bq_d = nc.dram_tensor("bq", [D], F32, kind="ExternalInput")
    bk_d = nc.dram_tensor("bk", [D], F32, kind="ExternalInput")
    bv_d = nc.dram_tensor("bv", [D], F32R, kind="ExternalInput")
    bo_d = nc.dram_tensor("bo", [D], F32, kind="ExternalInput")
    b1_d = nc.dram_tensor("b1", [DFF], F32, kind="ExternalInput")
    b2_d = nc.dram_tensor("b2", [D], F32, kind="ExternalInput")
    g1_d = nc.dram_tensor("ln1gb", [2, D], F32R, kind="ExternalInput")
    g2_d = nc.dram_tensor("ln2gb", [2, D], F32R, kind="ExternalInput")
    mAB_d = nc.dram_tensor("maskAB", [EB_KT * P, R], BF16, kind="ExternalInput")
    mAhi_d = nc.dram_tensor("maskAhi", [EB_KT * P, QW], BF16, kind="ExternalInput")
    out_d = nc.dram_tensor("outT", [D, R], F32, kind="ExternalOutput")

    xT_r = xT_d[:, :].rearrange("(t p) n -> p t n", p=P)
    xqT_r = xqT_d[:, :].rearrange("(t p) n -> p t n", p=P)
    out_r = out_d[:, :].rearrange("(t p) n -> p t n", p=P)

    with tile.TileContext(nc) as tc, ExitStack() as ctx:
        ctx.enter_context(nc.allow_low_precision(
            reason="float32r outputs feed fp32r matmuls by design"))
        pers = ctx.enter_context(tc.tile_pool(name="pers", bufs=1))

        # ---- constants / small persistent tiles
        ones128 = pers.tile([P, 1], F32R)
        nc.vector.memset(ones128.bitcast(F32), 1.0)
        ones_row = pers.tile([1, P], F32R)
        nc.vector.memset(ones_row.bitcast(F32), 1.0)
        eps1 = pers.tile([1, 1], F32)
        nc.vector.memset(eps1, 1e-5)

        def load_stack(t0, tag):
            st = pers.tile([2, D], F32R, tag=tag)
            nc.sync.dma_start(st, t0[:, :])
            return st

        g1 = load_stack(g1_d, 'g1')
        g2 = load_stack(g2_d, 'g2')

        def load_bias_cols(src, ntiles, tag):
            t = pers.tile([P, ntiles], F32, tag=tag)
            nc.sync.dma_start(t, src[:].rearrange("(t p) -> p t", p=P))
            return t

        bo_sb = load_bias_cols(bo_d, DT, 'bo')
        bq_sb = load_bias_cols(bq_d, DT, 'bq')
        bk_sb = load_bias_cols(bk_d, DT, 'bk')
        b2_sb = load_bias_cols(b2_d, DT, 'b2')
        b1_sb = load_bias_cols(b1_d, MT, 'b1')
        bv_row = pers.tile([1, D], F32R)
        nc.sync.dma_start(bv_row, bv_d[:].rearrange("(a d) -> a d", a=1))

        # ---- persistent activations (allocated up front, stack base)
        xnT = pers.tile([P, DT, T], BF16, tag="xnT_hg")       # 32KB/p, reused as hg
        xnqT = pers.tile([P, DT, R], BF16, tag="xnq_xn2")      # 16KB/p (shared w/ xn2T)
        KTs = pers.tile([P, DT, T], BF16)                      # 32KB/p
        Vs = pers.tile([P, KT, H * (HD + 1)], BF16)            # 32.5KB/p
        qTs = pers.tile([P, DT, R], BF16)                      # 8KB/p
        YTs = pers.tile([P, DT, R], BF16)                      # 8KB/p
        x1T = pers.tile([P, DT, R], F32)                       # 16KB/p

        # =============== transposed layernorm helper =================
        # Emits, per 512-token chunk: PE stats (ones-matmul sums), scalar
        # math, rank-1/2 broadcast matmuls, and the normalize ops; then
        # calls post_chunk(ch) so downstream consumers of the chunk can be
        # interleaved for PE density.
        def ln_T(get_src, ncols, gstack, write_dst, stats_f32r=True,
                 post_chunk=None):
            nch = ncols // 512
            with (
                tc.tile_pool(name="lnw", bufs=3) as lw,
                tc.tile_pool(name="lns", bufs=1) as lsm,
                tc.tile_pool(name="lnpa", bufs=1, space="PSUM") as pa,
                tc.tile_pool(name="lnpb", bufs=2, space="PSUM") as pb,
            ):
                for ch in range(nch):
                    sum_ps = pa.tile([1, 512], F32, tag="lnsum")
                    ssq_ps = pa.tile([1, 512], F32, tag="lnssq")
                    sdt = F32R if stats_f32r else F32
                    one_l = ones128 if stats_f32r else ones128.bitcast(F32)
                    for dt_i in range(DT):
                        xs = get_src(dt_i, ch)
                        sq = lw.tile([P, 512], sdt, tag="lnsq")
                        nc.vector.tensor_mul(sq, xs, xs)
                        nc.tensor.matmul(
                            sum_ps, one_l, xs,
                            start=(dt_i == 0), stop=(dt_i == DT - 1))
                        nc.tensor.matmul(
                            ssq_ps, one_l, sq[:, :],
                            start=(dt_i == 0), stop=(dt_i == DT - 1))
                    mu = lsm.tile([1, 512], F32, tag="mu")
                    nc.vector.tensor_scalar_mul(mu, sum_ps, 1.0 / D)
                    msq = lsm.tile([1, 512], F32, tag="msq")
                    nc.vector.tensor_scalar_mul(msq, ssq_ps, 1.0 / D)
                    musq = lsm.tile([1, 512], F32, tag="musq")
                    nc.vector.tensor_mul(musq, mu, mu)
                    nc.vector.tensor_sub(msq, msq, musq)
                    sd = lsm.tile([1, 512], F32, tag="sd")
                    nc.scalar.activation(sd, msq, AF.Sqrt, bias=eps1)
                    rsd = lsm.tile([1, 512], F32R, tag="rstd")
                    nc.vector.reciprocal(rsd, sd)
                    r2 = lsm.tile([2, 512], F32R, tag="r2")
                    nc.vector.memset(r2.bitcast(F32), 1.0)
                    nc.vector.scalar_tensor_tensor(
                        r2[0:1, :], mu, -1.0, rsd, OP.mult, OP.mult)
                    for dt_i in range(DT):
                        b1p = pb.tile([P, 512], F32, tag="lnb1")
                        nc.tensor.matmul(
                            b1p, gstack[0:1, ts(dt_i, P)],
                            rsd, start=True, stop=True)
                        b2p = pb.tile([P, 512], F32, tag="lnb2")
                        nc.tensor.matmul(
                            b2p, gstack[0:2, ts(dt_i, P)],
                            r2[:, :], start=True, stop=True)
                        xs = get_src(dt_i, ch)
                        t = lw.tile([P, 512], F32, tag="lnt")
                        nc.vector.tensor_mul(t, xs, b1p)
                        nc.vector.tensor_add(write_dst(dt_i, ch), t, b2p)
                    if post_chunk is not None:
                        post_chunk(ch)

        # =============== P1+P2: LN1 (streamed) interleaved with K-proj,
        # then Q and V projections ==========================================
        with (
            tc.tile_pool(name="xstream", bufs=6) as xsp,
            tc.tile_pool(name="wstream", bufs=1) as wp,
            tc.tile_pool(name="p2ps", bufs=2, space="PSUM") as pp,
        ):
            wk_sb = wp.tile([P, DT, D], BF16, tag="w")
            nc.sync.dma_start(wk_sb, wk_d[:, :].rearrange("(t p) m -> p t m", p=P))

            def src_xT(dt_i, ch):
                t = xsp.tile([P, 512], F32R, tag="xs")
                nc.sync.dma_start(t, xT_r[:, dt_i, ts(ch, 512)])
                return t

            def k_proj_chunk(chn):
                for dt_o in range(DT):
                    ps = pp.tile([P, 512], F32, tag="proj")
                    for kt_i in range(DT):
                        nc.tensor.matmul(
                            ps, wk_sb[:, kt_i, ts(dt_o, P)],
                            xnT[:, kt_i, ts(chn, 512)],
                            start=(kt_i == 0), stop=(kt_i == DT - 1))
                    nc.vector.tensor_scalar_add(KTs[:, dt_o, ts(chn, 512)], ps,
                                                bk_sb[:, dt_o:dt_o + 1])

            ln_T(src_xT, T, g1, lambda dt_i, ch: xnT[:, dt_i, ts(ch, 512)],
                 post_chunk=k_proj_chunk)

            # LN1 on query rows
            def src_xq(dt_i, ch):
                t = xsp.tile([P, 512], F32R, tag="xs")
                nc.sync.dma_start(t, xqT_r[:, dt_i, :])
                return t

            ln_T(src_xq, R, g1,
                 lambda dt_i, ch: xnqT[:, dt_i, ts(ch, 512)])

            # Q^T = Wq^T @ xnq^T (+bq)
            wq_sb = wp.tile([P, DT, D], BF16, tag="w")
            nc.sync.dma_start(wq_sb, wq_d[:, :].rearrange("(t p) m -> p t m", p=P))
            for dt_o in range(DT):
                ps = pp.tile([P, R], F32, tag="proj")
                for kt_i in range(DT):
                    nc.tensor.matmul(
                        ps, wq_sb[:, kt_i, ts(dt_o, P)], xnqT[:, kt_i, :],
                        start=(kt_i == 0), stop=(kt_i == DT - 1))
                nc.vector.tensor_scalar_add(qTs[:, dt_o, :], ps,
                                            bq_sb[:, dt_o:dt_o + 1])

            # bv broadcast across partitions
            bvb = wp.tile([P, D], BF16, tag="bvb")
            for chn in range(2):
                ps = pp.tile([P, 512], F32, tag="proj")
                nc.tensor.matmul(ps, ones_row,
                                 bv_row[0:1, ts(chn, 512)],
                                 start=True, stop=True)
                nc.vector.tensor_copy(bvb[:, ts(chn, 512)], ps)

            # V natural = xn @ Wv (+bv), per-head interleave + ones column
            for vch in range(2):
                wv_sb = wp.tile([P, DT, 512], BF16, tag="w")
                nc.sync.dma_start(
                    wv_sb,
                    wv_d[:, ts(vch, 512)].rearrange("(t p) m -> p t m", p=P))
                for kto in range(KT):
                    ps = pp.tile([P, 512], F32, tag="proj")
                    for kt_i in range(DT):
                        nc.tensor.matmul(
                            ps, xnT[:, kt_i, ts(kto, P)], wv_sb[:, kt_i, :],
                            start=(kt_i == 0), stop=(kt_i == DT - 1))
                    dst = Vs[:, kto, :].rearrange(
                        "p (h c) -> p h c", c=HD + 1)[:, 8 * vch:8 * vch + 8, 0:HD]
                    nc.vector.tensor_add(
                        dst, ps.rearrange("p (h c) -> p h c", c=HD),
                        bvb[:, ts(vch, 512)].rearrange("p (h c) -> p h c", c=HD))
            ones_col = Vs[:, :, :].rearrange(
                "p t (h c) -> p t h c", c=HD + 1)[:, :, :, HD:HD + 1]
            nc.vector.memset(ones_col, 1.0)

        # =============== P3: attention (sw-pipelined) + output proj ========
        with (
            tc.tile_pool(name="maskp", bufs=1) as mp,
            tc.tile_pool(name="attw", bufs=PIPE + 1) as aw,
            tc.tile_pool(name="attw2", bufs=2) as aw2,
            tc.tile_pool(name="atts", bufs=2) as asml,
            tc.tile_pool(name="wop", bufs=1) as wop,
            tc.tile_pool(name="ps3s", bufs=PIPE + 1, space="PSUM") as ps3s,
            tc.tile_pool(name="ps3y", bufs=2, space="PSUM") as ps3y,
            tc.tile_pool(name="ps3b", bufs=1, space="PSUM") as ps3b,
        ):
            mAB_sb = mp.tile([P, EB_KT, R], BF16)
            nc.sync.dma_start(mAB_sb, mAB_d[:, :].rearrange("(t p) q -> p t q", p=P))
            mAhi_sb = mp.tile([P, EB_KT, QW], BF16)
            nc.sync.dma_start(mAhi_sb, mAhi_d[:, :].rearrange("(t p) q -> p t q", p=P))
            for h in range(H):
                base = (h % 2) * 64
                dt_h = h // 2
                vsl = slice(h * (HD + 1), (h + 1) * (HD + 1))
                y_ps = ps3y.tile([HD + 1, R], F32, tag="ypsum")
                pbfs = {}

                def front(kt_i, base=base, dt_h=dt_h, pbfs=pbfs):
                    full = kt_i < EB_KT
                    qsl = slice(0, R) if full else slice(QW, R)
                    w = R if full else QW
                    s_ps = ps3s.tile([P, R], F32, tag="spsum")
                    nc.tensor.matmul(
                        s_ps[:, 0:w], KTs[base:base + 64, dt_h, ts(kt_i, P)],
                        qTs[base:base + 64, dt_h, qsl],
                        start=True, stop=True)
                    pbf = aw.tile([P, R], BF16, tag="pbf")
                    nc.scalar.activation(pbf[:, 0:w], s_ps[:, 0:w], AF.Exp,
                                         scale=0.125)
                    m_ap = (mAB_sb[:, kt_i, :] if full
                            else mAhi_sb[:, kt_i - EB_KT, :])
                    nc.vector.tensor_mul(pbf[:, 0:w], pbf[:, 0:w], m_ap)
                    pbfs[kt_i] = pbf

                def av(kt_i, y_ps=y_ps, vsl=vsl, pbfs=pbfs):
                    full = kt_i < EB_KT
                    osl = slice(0, R) if full else slice(QW, R)
                    w = R if full else QW
                    nc.tensor.matmul(
                        y_ps[:, osl], Vs[:, kt_i, vsl], pbfs.pop(kt_i)[:, 0:w],
                        start=(kt_i == 0), stop=(kt_i == EA_KT - 1),
                        skip_group_check=True)

                for kt_i in range(PIPE):
                    front(kt_i)
                for kt_i in range(EA_KT):
                    if kt_i + PIPE < EA_KT:
                        front(kt_i + PIPE)
                    av(kt_i)

                rec = asml.tile([1, R], F32R, tag="rec")
                nc.vector.reciprocal(rec, y_ps[HD:HD + 1, :])
                b_ps = ps3b.tile([64, R], F32, tag="bps")
                nc.tensor.matmul(b_ps, ones_row[0:1, 0:64],
                                 rec, start=True, stop=True)
                b_sb = aw2.tile([64, R], F32, tag="bsb")
                nc.vector.tensor_copy(b_sb, b_ps)
                nc.vector.tensor_mul(
                    YTs[base:base + 64, dt_h, :], y_ps[0:HD, :], b_sb)

            # output projection + residual
            wo_sb = wop.tile([P, DT, D], BF16)
            nc.sync.dma_start(wo_sb, wo_d[:, :].rearrange("(t p) m -> p t m", p=P))
            for dt_o in range(DT):
                ps = ps3s.tile([P, R], F32, tag="spsum")
                for kt_i in range(DT):
                    nc.tensor.matmul(
                        ps, wo_sb[:, kt_i, ts(dt_o, P)], YTs[:, kt_i, :],
                        start=(kt_i == 0), stop=(kt_i == DT - 1))
                xq4 = aw2.tile([P, R], F32R, tag="xq4")
                nc.sync.dma_start(xq4, xqT_r[:, dt_o, :])
                nc.vector.scalar_tensor_tensor(
                    x1T[:, dt_o, :], ps, bo_sb[:, dt_o:dt_o + 1],
                    xq4, OP.add, OP.add)

        # =============== P5: LN2 ===========================================
        xn2T = pers.tile([P, DT, R], BF16, tag="xnq_xn2")
        ln_T(lambda dt_i, ch: x1T[:, dt_i, ts(ch, 512)], R, g2,
             lambda dt_i, ch: xn2T[:, dt_i, ts(ch, 512)], stats_f32r=False)

        # =============== P6: fc1 + gelu ====================================
        hg = pers.tile([P, MT, R], BF16, tag="xnT_hg")
        with (
            tc.tile_pool(name="w1p", bufs=3) as wp1,
            tc.tile_pool(name="p6ps", bufs=4, space="PSUM") as pp6,
        ):
            for mt in range(MT):
                w1_sb = wp1.tile([P, DT, P], BF16, tag="w1")
                nc.sync.dma_start(w1_sb, w1_d[mt])
                ps = pp6.tile([P, R], F32, tag="proj")
                for kt_i in range(DT):
                    nc.tensor.matmul(
                        ps, w1_sb[:, kt_i, :], xn2T[:, kt_i, :],
                        start=(kt_i == 0), stop=(kt_i == DT - 1))
                nc.scalar.activation(hg[:, mt, :], ps, AF.Gelu,
                                     bias=b1_sb[:, mt:mt + 1])

        # =============== P7: fc2 + bias + residual + store =================
        with (
            tc.tile_pool(name="w2p", bufs=2) as wp2,
            tc.tile_pool(name="outp", bufs=2) as op_,
            tc.tile_pool(name="p7ps", bufs=4, space="PSUM") as pp7,
        ):
            for dt_o in range(DT):
                w2_sb = wp2.tile([P, MT, P], BF16, tag="w2")
                nc.sync.dma_start(w2_sb, w2_d[dt_o])
                ps = pp7.tile([P, R], F32, tag="proj")
                for kt_i in range(MT):
                    nc.tensor.matmul(
                        ps, w2_sb[:, kt_i, :], hg[:, kt_i, :],
                        start=(kt_i == 0), stop=(kt_i == MT - 1))
                out_sb = op_.tile([P, R], F32, tag="out")
                nc.vector.scalar_tensor_tensor(
                    out_sb, ps, b2_sb[:, dt_o:dt_o + 1], x1T[:, dt_o, :],
                    OP.add, OP.add)
                nc.sync.dma_start(out_r[:, dt_o, :], out_sb)

    nc.compile()
    return nc


def _prep_in_maps(inputs):
    bf = ml_dtypes.bfloat16
    x = np.asarray(inputs["x"], np.float32)
    w1 = np.asarray(inputs["W1"], np.float32).astype(bf)   # [D, DFF]
    w2 = np.asarray(inputs["W2"], np.float32).astype(bf)   # [DFF, D]
    # packed layouts for contiguous weight-tile DMA:
    # w1p[mt, p, kt, m] = W1[kt*128+p, mt*128+m]
    w1p = np.ascontiguousarray(
        w1.reshape(DT, P, MT, P).transpose(2, 1, 0, 3))
    # w2p[dt, p, kt, m] = W2[kt*128+p, dt*128+m]
    w2p = np.ascontiguousarray(
        w2.reshape(MT, P, DT, P).transpose(2, 1, 0, 3))
    shared = {
        "wq": np.asarray(inputs["Wq"], np.float32).astype(bf),
        "wk": np.asarray(inputs["Wk"], np.float32).astype(bf),
        "wv": np.asarray(inputs["Wv"], np.float32).astype(bf),
        "wo": np.asarray(inputs["Wo"], np.float32).astype(bf),
        "w1p": w1p,
        "w2p": w2p,
        "bq": np.asarray(inputs["bq"], np.float32),
        "bk": np.asarray(inputs["bk"], np.float32),
        "bv": np.asarray(inputs["bv"], np.float32),
        "bo": np.asarray(inputs["bo"], np.float32),
        "b1": np.asarray(inputs["b1"], np.float32),
        "b2": np.asarray(inputs["b2"], np.float32),
        "ln1gb": np.stack([np.asarray(inputs["ln1_g"], np.float32),
                           np.asarray(inputs["ln1_b"], np.float32)]),
        "ln2gb": np.stack([np.asarray(inputs["ln2_g"], np.float32),
                           np.asarray(inputs["ln2_b"], np.float32)]),
    }
    in_maps = []
    for c in range(N_CORES):
        b, i = divmod(c, 4)
        lo = slice(QW * i, QW * i + QW)
        hi = slice(QW * (7 - i), QW * (8 - i))
        xb = x[b]                                    # [T, D]
        xq = np.concatenate([xb[lo], xb[hi]], 0)     # [R, D] (lo | hi)
        rows_lo = np.arange(QW * i, QW * i + QW)
        rows_hi = np.arange(QW * (7 - i), QW * (8 - i))
        kA = np.arange(T)[:, None]
        kB = np.arange(EB_KT * P)[:, None]
        maskA = (kA <= rows_hi[None, :]).astype(bf)
        maskB = (kB <= rows_lo[None, :]).astype(bf)
        maskAB = np.concatenate([maskB, maskA[:EB_KT * P]], axis=1)  # [1024, 512]
        in_maps.append({
            **shared,
            "xT": np.ascontiguousarray(xb.T),
            "xqT": np.ascontiguousarray(xq.T),
            "maskAB": maskAB,
            "maskAhi": np.ascontiguousarray(maskA[EB_KT * P:]),
        })
    return in_maps


_PROGRAM = None


def _get_program():
    global _PROGRAM
    if _PROGRAM is None:
        _PROGRAM = _build_program()
    return _PROGRAM


def _assemble(results):
    out = np.empty((B, T, D), np.float32)
    for c in range(N_CORES):
        b, i = divmod(c, 4)
        oT = np.asarray(results[c]["outT"], np.float32)   # [D, R]
        out[b, QW * i:QW * i + QW] = oT[:, :QW].T
        out[b, QW * (7 - i):QW * (8 - i)] = oT[:, QW:].T
    return out


def kernel(**inputs):
    from concourse.bass_utils import run_bass_kernel_spmd
    nc = _get_program()
    in_maps = _prep_in_maps(inputs)
    res = run_bass_kernel_spmd(nc, in_maps, list(range(N_CORES)))
    return _assemble(res.results)
